# revision 22
# baseline (speedup 1.0000x reference)
"""Trainium2 Bass kernel v7 for nn_Attention_30666066493686.

Region-attention over N=36 regions:
  hidden = tanh(region @ Wr + frame @ Wf + b_att)          [T,N,B,A]
  att    = hidden . W_full  (+ b_full, dropped: softmax-shift invariant)
  alpha  = softmax_n(where(mask, -1e9, att))
  out    = sum_n alpha * region                            [T,B,D]

Sharding: data-parallel over T across 8 NeuronCores (4 timesteps each);
params replicated; no collectives.

v7 = v6 + trace-driven fixes:
  - rT loads split j-major ([:, 0:2, :] / [:, 2:4, :]) so each DMA
    descriptor covers 2560 contiguous bytes per partition (v6's rc-major
    split produced 640B descriptors, 75% engine efficiency).
  - rN loads in 2 pieces (5 chunks each -> 5KB descriptors).
  - out stores in bf16 on the scalar (ACT) HWDGE ring, issued right
    after each osb: the SP ring carries only loads, so the next
    iteration's region loads are never head-of-line-blocked behind
    stores waiting on p2 compute. Host casts back to f32.
  - p1 weight hoist: jp-outer loop loads w8[jp] once per timestep and
    streams all three PSUM groups (v6 reloaded weights per group: 9
    LDWEIGHTS/t -> 3).
  - dg (diag-expanded alpha weights) built in ONE DVE tensor_tensor per
    timestep with a broadcast AP (v6: 10 tensor_scalar ops/t, each with
    ~150ns fixed cost).
  - hardware loop unrolled 8x (v6: 4x) to amortize the For_i end-of-body
    drain + semaphore-reset barrier (~2us of DMA idle per body).

v8: the masked-softmax denominator no longer uses 10 small PE matmuls
per timestep (each cost ~110ns of issue spacing; PE was the bottleneck
at 91% busy). Instead: fold dg over chunks on DVE (strided
tensor_reduce) to S_rows[128, 64], then one [128,64]x[128,1] ones
matmul gives S[64, 1]. The expb bf16 cast dies with it.

v9: in-body the DMA is 97% saturated at ~24.3us/copy, but the For_i
body boundary drains the whole pipeline (~2.4us/iter at unroll=8).
  - unroll 16 halves the barrier share (32 was tried and is WORSE:
    24.9us vs 24.0 — bigger bodies hurt dispatch).
  - The rT (phase-1 fp8) stream is trimmed to the exact per-t-slot row
    count (rcw_t = max kept rows over cores, ceil-16): the ~7% tail of
    all-zero padding rows is no longer shipped or streamed through the
    PE. The th columns beyond rcw_t become a ghost zone: the feeding
    PSUM region is memset to 0 so tanh yields finite junk that the
    zero indicator columns (indB) nullify exactly.
  - One dma_start per (t, stream): 5KB rT descriptors, 10KB rN.

v11 (final, 21.6us vs 29.8us baseline): the rN tail chunk is ~75%
zero-padding rows — ship only ceil32(rcw-1152) partitions and zero the
rest with DVE memsets (32-partition pieces; NOT gpsimd memset, whose
SWDGE path cost +5us). Beyond the ~1us of DMA bytes this also lands
the tail chunk early, shortening the pipeline tail at the body
boundary: measured -2.7us. The kernel is now at ~97% of the per-core
HBM roofline (7.56 MB/iter at ~358 GB/s).
"""

import ml_dtypes
import numpy as np

T, N, B, D, A = 32, 36, 64, 512, 128
N_CORES = 8
T_LOC = T // N_CORES           # 4
ROWS = N * B                   # 2304
NJ = D // 128                  # 4

# const blob column layout (bf16): Wf | wfull | frameT | b_att
CB_WF = 0
CB_WFULL = 512
CB_FRAMET = 513
CB_BATT = 1537
CB_W = 1538

_NC_CACHE = {}


def _groups(rc):
    out = []
    c0 = 0
    while c0 < rc:
        out.append((c0, min(512, rc - c0)))
        c0 += 512
    return out


def _build_nc(iters=1, nchc=10, unroll=1, rcws=None):
    import concourse.bacc as bacc
    from concourse import mybir
    from concourse.tile import TileContext

    f32 = mybir.dt.float32
    bf16 = mybir.dt.bfloat16
    fp8 = mybir.dt.float8e4
    AF = mybir.ActivationFunctionType
    rc = nchc * 128
    if rcws is None:
        rcws = (rc,) * T_LOC
    rcws = tuple(rcws)
    oT = [NJ * sum(rcws[:t]) for t in range(T_LOC)]  # flat rT offsets

    nc = bacc.Bacc(
        "TRN2", target_bir_lowering=False, debug=False, num_devices=N_CORES
    )
    regionTc = nc.dram_tensor("regionTc", [128, NJ * sum(rcws)], fp8, kind="ExternalInput")
    regionNc = nc.dram_tensor("regionNc", [T_LOC, 128, nchc * 512], bf16, kind="ExternalInput")
    cb16 = nc.dram_tensor("cb16", [128, CB_W], bf16, kind="ExternalInput")
    watt8 = nc.dram_tensor("watt8", [128, 512], fp8, kind="ExternalInput")
    indB = nc.dram_tensor("indB", [128, T_LOC * nchc * 64], fp8, kind="ExternalInput")
    ind2 = nc.dram_tensor("ind2", [64, T_LOC * rc], fp8, kind="ExternalInput")
    out = nc.dram_tensor("out", [T_LOC, B, D], bf16, kind="ExternalOutput")

    with TileContext(nc) as tc:
        with (
            tc.tile_pool(name="consts", bufs=1) as consts,
            tc.tile_pool(name="rtp", bufs=8) as rtp,
            tc.tile_pool(name="rnp", bufs=8) as rnp,
            tc.tile_pool(name="thp", bufs=2) as thp,
            tc.tile_pool(name="smallp", bufs=4) as smallp,
            tc.tile_pool(name="diagp", bufs=3) as diagp,
            tc.tile_pool(name="outp", bufs=4) as outp,
            tc.tile_pool(name="phh", bufs=4, space="PSUM") as phh,
            tc.tile_pool(name="psmall", bufs=2, space="PSUM") as psmall,
            tc.tile_pool(name="po", bufs=2, space="PSUM") as po,
        ):
            # ---- constants: packed DMAs on the ACT queue ----
            cb = consts.tile([128, CB_W], bf16)
            nc.scalar.dma_start(out=cb, in_=cb16.ap())
            w8 = consts.tile([128, 2, 2, 128], fp8)
            nc.scalar.dma_start(
                out=w8, in_=watt8.ap().rearrange("p (kp kk a) -> p kp kk a", kp=2, kk=2)
            )
            indB_sb = consts.tile([128, T_LOC, nchc, 64], fp8)
            nc.scalar.dma_start(
                out=indB_sb,
                in_=indB.ap().rearrange("p (t c b) -> p t c b", t=T_LOC, c=nchc),
            )
            ind2_sb = consts.tile([64, T_LOC, rc], fp8)
            nc.scalar.dma_start(
                out=ind2_sb, in_=ind2.ap().rearrange("p (t r) -> p t r", t=T_LOC)
            )

            def wf_sb(j):  # [128, 128] chunk j of Wf (frame half of W_att)
                return cb[:, CB_WF + j * 128 : CB_WF + (j + 1) * 128]

            wfull_sb = cb[:, CB_WFULL : CB_WFULL + 1]

            def frameT_sb(j):
                return cb[:, CB_FRAMET + j * 256 : CB_FRAMET + (j + 1) * 256]

            batt_col = cb[:, CB_BATT : CB_BATT + 1]

            ones_col = consts.tile([128, 1], f32)
            nc.vector.memset(ones_col, 1.0)

            # ---- preamble: fproj[b, A] per t = (frame @ Wf)[t] ----
            fproj_sb = consts.tile([64, T_LOC, 128], bf16)
            for t in range(T_LOC):
                pf = psmall.tile([64, 128], f32, tag="s", name=f"pf{t}")
                for j in range(NJ):
                    nc.tensor.matmul(
                        pf,
                        lhsT=frameT_sb(j)[:, t * 64 : (t + 1) * 64],
                        rhs=wf_sb(j),
                        start=(j == 0),
                        stop=(j == NJ - 1),
                    )
                nc.scalar.copy(out=fproj_sb[:, t, :], in_=pf)

            # ---- per-timestep body ----
            def body(_iv=None, copies=1):
                per = [({}, {}, {}, []) for _ in range(copies)]

                def load_rT(st, t):
                    rTs, rNs, states, osbs = st
                    rcw = rcws[t]
                    rT = rtp.tile([128, NJ, rcw], fp8, tag="rT", name=f"rT{t}")
                    rTd = regionTc.ap()[:, oT[t] : oT[t] + NJ * rcw].rearrange(
                        "p (j r) -> p j r", j=NJ
                    )
                    nc.sync.dma_start(out=rT, in_=rTd)
                    rTs[t] = rT

                def load_rN(st, t):
                    rTs, rNs, states, osbs = st
                    rN = rnp.tile([128, nchc, 512], bf16, tag="rN", name=f"rN{t}")
                    rNd = regionNc.ap()[t].rearrange("p (c d) -> p c d", c=nchc)
                    # tail chunk is mostly zero-padding rows: ship only the
                    # real ones; zero the rest with DVE memsets (32-wide
                    # pieces for the partition-access rule)
                    ktail = min(128, max(0, rcws[t] - 128 * (nchc - 1)))
                    kload = min(128, -(-ktail // 32) * 32)
                    if kload < 128:
                        for pz in range(kload, 128, 32):
                            nc.vector.memset(rN[pz : pz + 32, nchc - 1, :], 0)
                        nc.sync.dma_start(
                            out=rN[:, : nchc - 1, :], in_=rNd[:, : nchc - 1, :]
                        )
                        if kload > 0:
                            nc.sync.dma_start(
                                out=rN[:kload, nchc - 1, :],
                                in_=rNd[:kload, nchc - 1, :],
                            )
                    else:
                        nc.sync.dma_start(out=rN, in_=rNd)
                    rNs[t] = rN

                def p1(st, t):
                    rTs, rNs, states, osbs = st
                    # phase 1: hidden^T[A, rows] = Wr^T @ region^T (fp8
                    # DoubleRow) + fproj routed via ind2; tanh adds b_att.
                    # jp-outer: one LDWEIGHTS per contraction slice, all
                    # PSUM groups streamed under the same weights.
                    # Matmuls only stream the exact rcw columns; the ghost
                    # zone [rcw, rc) of the last PSUM group is memset so
                    # tanh writes finite junk there (nullified by indB=0).
                    rT = rTs[t]
                    rcw = rcws[t]
                    gs = _groups(rc)
                    mws = [max(0, min(cw, rcw - c0)) for c0, cw in gs]
                    phs = [
                        phh.tile([128, 512], f32, tag="phh", name=f"ph{t}_{g}")
                        for g in range(len(gs))
                    ]
                    for jp in range(2):
                        for g, (c0, cw) in enumerate(gs):
                            if mws[g] == 0:
                                continue
                            nc.tensor.matmul(
                                phs[g][:, : mws[g]],
                                lhsT=w8[:, jp],
                                rhs=rT[:, 2 * jp : 2 * jp + 2, c0 : c0 + mws[g]],
                                start=(jp == 0),
                                stop=False,
                                perf_mode=mybir.MatmulPerfMode.DoubleRow,
                            )
                    for g, (c0, cw) in enumerate(gs):
                        if mws[g] == 0:
                            continue
                        nc.tensor.matmul(
                            phs[g][:, : mws[g]],
                            lhsT=fproj_sb[:, t, :],
                            rhs=ind2_sb[:, t, c0 : c0 + mws[g]],
                            start=False,
                            stop=True,
                        )
                    th = thp.tile([128, rc], bf16, tag="th", name=f"th{t}")
                    for g, (c0, cw) in enumerate(gs):
                        if mws[g] < cw:
                            nc.vector.memset(phs[g][:, mws[g] : cw], 0.0)
                        nc.scalar.activation(
                            out=th[:, c0 : c0 + cw],
                            in_=phs[g][:, :cw],
                            func=AF.Tanh,
                            bias=batt_col,
                        )
                    # att columns + masked-softmax denominator
                    patt = psmall.tile([128, nchc], f32, tag="s", name=f"pa{t}")
                    for c in range(nchc):
                        nc.tensor.matmul(
                            patt[:, c : c + 1],
                            lhsT=th[:, c * 128 : (c + 1) * 128],
                            rhs=wfull_sb,
                            start=True,
                            stop=True,
                        )
                    expf = smallp.tile([128, nchc], f32, tag="expf", name=f"ef{t}")
                    nc.scalar.activation(out=expf, in_=patt, func=AF.Exp)
                    states[t] = expf

                def p2(st, t):
                    rTs, rNs, states, osbs = st
                    # phase 2: out[b, :] = (sum_c diag-expand(e)_c^T @ rN_c)/S
                    expf = states[t]
                    rN = rNs[t]
                    # all nchc diag blocks in one DVE op (broadcast AP)
                    dg = diagp.tile([128, nchc, 64], bf16, tag="dg", name=f"dg{t}")
                    nc.vector.tensor_mul(
                        out=dg,
                        in0=indB_sb[:, t],
                        in1=expf[:, :, None].broadcast_to([128, nchc, 64]),
                    )
                    # denominator: fold dg over chunks (strided DVE
                    # reduce), then one ones-matmul folds partitions
                    srows = smallp.tile([128, 64], f32, tag="sr", name=f"sr{t}")
                    nc.vector.tensor_reduce(
                        out=srows[:, :, None],
                        in_=dg.transpose([0, 2, 1]),
                        axis=mybir.AxisListType.X,
                        op=mybir.AluOpType.add,
                    )
                    po_t = po.tile([64, 512], f32, tag="po", name=f"po{t}")
                    for c in range(nchc):
                        nc.tensor.matmul(
                            po_t,
                            lhsT=dg[:, c, :],
                            rhs=rN[:, c, :],
                            start=(c == 0),
                            stop=(c == nchc - 1),
                        )
                    psS = psmall.tile([64, 1], f32, tag="s", name=f"ps{t}")
                    nc.tensor.matmul(psS, lhsT=srows, rhs=ones_col, start=True, stop=True)
                    rs = smallp.tile([64, 1], f32, tag="rs", name=f"rs{t}")
                    nc.vector.reciprocal(out=rs, in_=psS)
                    osb = outp.tile([64, 512], bf16, tag="osb", name=f"ob{t}")
                    nc.scalar.activation(out=osb, in_=po_t, func=AF.Copy, scale=rs)
                    # store on the ACT HWDGE ring: load FIFO stays clean
                    nc.scalar.dma_start(out=out.ap()[t], in_=osb)

                for u, st in enumerate(per):
                    for t in range(T_LOC):
                        load_rT(st, t)
                    for t in range(T_LOC):
                        load_rN(st, t)
                for st in per:
                    for t in range(T_LOC):
                        p1(st, t)
                    for t in range(T_LOC):
                        p2(st, t)

            if iters == 1:
                body()
            else:
                assert iters % unroll == 0, (iters, unroll)
                with tc.For_i(
                    0, iters // unroll, 1, hint_engines=(mybir.EngineType.PE,)
                ) as iv:
                    body(iv, copies=unroll)

    nc.compile()
    return nc


def _get_nc(iters=1, nchc=10, unroll=None, rcws=None):
    if unroll is None:
        unroll = 16 if iters > 1 and iters % 16 == 0 else 1
    key = (iters, nchc, unroll, rcws)
    if key not in _NC_CACHE:
        _NC_CACHE[key] = _build_nc(iters, nchc, unroll, rcws)
    return _NC_CACHE[key]


def _nchc_for(mask):
    keep = ~np.asarray(mask, bool).reshape(T, ROWS)
    counts = keep.sum(axis=1)
    return max(1, int(-(-int(counts.max()) // 128)))


def _rcws_for(mask, nchc=None):
    """Exact per-t-slot row width: max kept rows over cores, ceil-16."""
    if nchc is None:
        nchc = _nchc_for(mask)
    rc = nchc * 128
    keep = ~np.asarray(mask, bool).reshape(N_CORES, T_LOC, ROWS)
    counts = keep.sum(axis=2).max(axis=0)  # [T_LOC]
    return tuple(int(min(rc, -(-int(c) // 16) * 16)) for c in counts)


def _make_in_maps(
    region_feat, frame_feat, mask, W_att, b_att, W_full, nchc=None, rcws=None
):
    bf16 = ml_dtypes.bfloat16
    fp8 = ml_dtypes.float8_e4m3
    mask = np.asarray(mask, bool)
    if nchc is None:
        nchc = _nchc_for(mask)
    if rcws is None:
        rcws = _rcws_for(mask, nchc)
    rc = nchc * 128

    region_f = np.asarray(region_feat, np.float32)        # [T, N, B, D]
    frame_b = np.asarray(frame_feat).astype(bf16)         # [T, B, D]

    w8 = np.ascontiguousarray(
        np.asarray(W_att)[:512]
        .astype(fp8)
        .reshape(4, 128, 128)
        .transpose(1, 0, 2)
        .reshape(128, 512)
    )

    in_maps = []
    for cidx in range(N_CORES):
        sl = slice(cidx * T_LOC, (cidx + 1) * T_LOC)
        regc = np.zeros((T_LOC, rc, D), np.float32)
        ohe = np.zeros((T_LOC, rc, 64), np.float32)
        for tt in range(T_LOC):
            kept = np.flatnonzero(~mask[sl][tt].reshape(ROWS))
            nk = len(kept)
            regc[tt, :nk] = region_f[sl][tt].reshape(ROWS, D)[kept]
            ohe[tt, np.arange(nk), kept % 64] = 1.0
        regT = np.concatenate(
            [
                regc[tt, : rcws[tt]]
                .astype(fp8)
                .reshape(rcws[tt], NJ, 128)
                .transpose(2, 1, 0)
                .reshape(128, NJ * rcws[tt])
                for tt in range(T_LOC)
            ],
            axis=1,
        )
        regT = np.ascontiguousarray(regT)
        regN = np.ascontiguousarray(
            regc.astype(bf16)
            .reshape(T_LOC, nchc, 128, 512)
            .transpose(0, 2, 1, 3)
            .reshape(T_LOC, 128, nchc * 512)
        )
        iB = np.ascontiguousarray(
            ohe.reshape(T_LOC, nchc, 128, 64)
            .transpose(2, 0, 1, 3)
            .reshape(128, T_LOC * nchc * 64)
        ).astype(fp8)
        i2 = np.ascontiguousarray(
            ohe.transpose(2, 0, 1).reshape(64, T_LOC * rc)
        ).astype(fp8)
        frm = frame_b[sl].reshape(T_LOC * B, NJ, 128)     # [tb, j, dd]
        frmT = frm.transpose(2, 1, 0).reshape(128, NJ * 256)
        cb = np.zeros((128, CB_W), bf16)
        cb[:, CB_WF : CB_WF + 512] = (
            np.asarray(W_att)[512:].reshape(4, 128, 128).transpose(1, 0, 2).reshape(128, 512)
        ).astype(bf16)
        cb[:, CB_WFULL] = np.asarray(W_full).astype(bf16)
        cb[:, CB_FRAMET : CB_FRAMET + 1024] = frmT
        cb[:, CB_BATT] = np.asarray(b_att).astype(bf16)
        in_maps.append(
            {
                "regionTc": regT,
                "regionNc": regN,
                "cb16": np.ascontiguousarray(cb),
                "watt8": w8,
                "indB": iB,
                "ind2": i2,
            }
        )
    return in_maps


def kernel(region_feat, frame_feat, mask, W_att, b_att, W_full, b_full=None):
    """Full-input entry point. b_full is accepted but unused: softmax is
    invariant to a constant shift of the logits."""
    from concourse.bass_utils import run_bass_kernel_spmd

    nchc = _nchc_for(mask)
    rcws = _rcws_for(mask, nchc)
    nc = _get_nc(1, nchc, rcws=rcws)
    in_maps = _make_in_maps(
        region_feat, frame_feat, mask, W_att, b_att, W_full, nchc=nchc, rcws=rcws
    )
    res = run_bass_kernel_spmd(nc, in_maps, core_ids=list(range(N_CORES)))
    return np.concatenate(
        [res.results[c]["out"] for c in range(N_CORES)], axis=0
    ).astype(np.float32)


# revision 23
# speedup vs baseline: 1.2340x; 1.2340x over previous
"""Trainium2 Bass kernel v7 for nn_Attention_30666066493686.

Region-attention over N=36 regions:
  hidden = tanh(region @ Wr + frame @ Wf + b_att)          [T,N,B,A]
  att    = hidden . W_full  (+ b_full, dropped: softmax-shift invariant)
  alpha  = softmax_n(where(mask, -1e9, att))
  out    = sum_n alpha * region                            [T,B,D]

Sharding: data-parallel over T across 8 NeuronCores (4 timesteps each);
params replicated; no collectives.

v7 = v6 + trace-driven fixes:
  - rT loads split j-major ([:, 0:2, :] / [:, 2:4, :]) so each DMA
    descriptor covers 2560 contiguous bytes per partition (v6's rc-major
    split produced 640B descriptors, 75% engine efficiency).
  - rN loads in 2 pieces (5 chunks each -> 5KB descriptors).
  - out stores in bf16 on the scalar (ACT) HWDGE ring, issued right
    after each osb: the SP ring carries only loads, so the next
    iteration's region loads are never head-of-line-blocked behind
    stores waiting on p2 compute. Host casts back to f32.
  - p1 weight hoist: jp-outer loop loads w8[jp] once per timestep and
    streams all three PSUM groups (v6 reloaded weights per group: 9
    LDWEIGHTS/t -> 3).
  - dg (diag-expanded alpha weights) built in ONE DVE tensor_tensor per
    timestep with a broadcast AP (v6: 10 tensor_scalar ops/t, each with
    ~150ns fixed cost).
  - hardware loop unrolled 8x (v6: 4x) to amortize the For_i end-of-body
    drain + semaphore-reset barrier (~2us of DMA idle per body).

v8: the masked-softmax denominator no longer uses 10 small PE matmuls
per timestep (each cost ~110ns of issue spacing; PE was the bottleneck
at 91% busy). Instead: fold dg over chunks on DVE (strided
tensor_reduce) to S_rows[128, 64], then one [128,64]x[128,1] ones
matmul gives S[64, 1]. The expb bf16 cast dies with it.

v9: in-body the DMA is 97% saturated at ~24.3us/copy, but the For_i
body boundary drains the whole pipeline (~2.4us/iter at unroll=8).
  - unroll 16 halves the barrier share (32 was tried and is WORSE:
    24.9us vs 24.0 — bigger bodies hurt dispatch).
  - The rT (phase-1 fp8) stream is trimmed to the exact per-t-slot row
    count (rcw_t = max kept rows over cores, ceil-16): the ~7% tail of
    all-zero padding rows is no longer shipped or streamed through the
    PE. The th columns beyond rcw_t become a ghost zone: the feeding
    PSUM region is memset to 0 so tanh yields finite junk that the
    zero indicator columns (indB) nullify exactly.
  - One dma_start per (t, stream): 5KB rT descriptors, 10KB rN.

v11 (final, 21.6us vs 29.8us baseline): the rN tail chunk is ~75%
zero-padding rows — ship only ceil32(rcw-1152) partitions and zero the
rest with DVE memsets (32-partition pieces; NOT gpsimd memset, whose
SWDGE path cost +5us). Beyond the ~1us of DMA bytes this also lands
the tail chunk early, shortening the pipeline tail at the body
boundary: measured -2.7us. The kernel is now at ~97% of the per-core
HBM roofline (7.56 MB/iter at ~358 GB/s).
"""

import ml_dtypes
import numpy as np

T, N, B, D, A = 32, 36, 64, 512, 128
N_CORES = 8
T_LOC = T // N_CORES           # 4
ROWS = N * B                   # 2304
NJ = D // 128                  # 4

# const blob column layout (bf16): Wf | wfull | frameT | b_att
CB_WF = 0
CB_WFULL = 512
CB_FRAMET = 513
CB_BATT = 1537
CB_W = 1538

_NC_CACHE = {}


def _groups(rc):
    out = []
    c0 = 0
    while c0 < rc:
        out.append((c0, min(512, rc - c0)))
        c0 += 512
    return out


def _build_nc(iters=1, nchc=10, unroll=1, rcws=None):
    import concourse.bacc as bacc
    from concourse import mybir
    from concourse.tile import TileContext

    f32 = mybir.dt.float32
    bf16 = mybir.dt.bfloat16
    fp8 = mybir.dt.float8e4
    AF = mybir.ActivationFunctionType
    rc = nchc * 128
    if rcws is None:
        rcws = (rc,) * T_LOC
    rcws = tuple(rcws)
    oT = [NJ * sum(rcws[:t]) for t in range(T_LOC)]  # flat rT offsets

    nc = bacc.Bacc(
        "TRN2", target_bir_lowering=False, debug=False, num_devices=N_CORES
    )
    regionTc = nc.dram_tensor("regionTc", [128, NJ * sum(rcws)], fp8, kind="ExternalInput")
    regionNc = nc.dram_tensor("regionNc", [T_LOC, 128, nchc * 512], bf16, kind="ExternalInput")
    cb16 = nc.dram_tensor("cb16", [128, CB_W], bf16, kind="ExternalInput")
    watt8 = nc.dram_tensor("watt8", [128, 512], fp8, kind="ExternalInput")
    indB = nc.dram_tensor("indB", [128, T_LOC * nchc * 64], fp8, kind="ExternalInput")
    ind2 = nc.dram_tensor("ind2", [64, T_LOC * rc], fp8, kind="ExternalInput")
    out = nc.dram_tensor("out", [T_LOC, B, D], bf16, kind="ExternalOutput")

    with TileContext(nc) as tc:
        with (
            tc.tile_pool(name="consts", bufs=1) as consts,
            tc.tile_pool(name="rtp", bufs=8) as rtp,
            tc.tile_pool(name="rnp", bufs=8) as rnp,
            tc.tile_pool(name="thp", bufs=2) as thp,
            tc.tile_pool(name="smallp", bufs=4) as smallp,
            tc.tile_pool(name="diagp", bufs=3) as diagp,
            tc.tile_pool(name="outp", bufs=4) as outp,
            tc.tile_pool(name="phh", bufs=4, space="PSUM") as phh,
            tc.tile_pool(name="psmall", bufs=2, space="PSUM") as psmall,
            tc.tile_pool(name="po", bufs=2, space="PSUM") as po,
        ):
            # ---- constants: packed DMAs on the ACT queue ----
            cb = consts.tile([128, CB_W], bf16)
            nc.scalar.dma_start(out=cb, in_=cb16.ap())
            w8 = consts.tile([128, 2, 2, 128], fp8)
            nc.scalar.dma_start(
                out=w8, in_=watt8.ap().rearrange("p (kp kk a) -> p kp kk a", kp=2, kk=2)
            )
            indB_sb = consts.tile([128, T_LOC, nchc, 64], fp8)
            nc.scalar.dma_start(
                out=indB_sb,
                in_=indB.ap().rearrange("p (t c b) -> p t c b", t=T_LOC, c=nchc),
            )
            ind2_sb = consts.tile([64, T_LOC, rc], fp8)
            nc.scalar.dma_start(
                out=ind2_sb, in_=ind2.ap().rearrange("p (t r) -> p t r", t=T_LOC)
            )

            def wf_sb(j):  # [128, 128] chunk j of Wf (frame half of W_att)
                return cb[:, CB_WF + j * 128 : CB_WF + (j + 1) * 128]

            wfull_sb = cb[:, CB_WFULL : CB_WFULL + 1]

            def frameT_sb(j):
                return cb[:, CB_FRAMET + j * 256 : CB_FRAMET + (j + 1) * 256]

            batt_col = cb[:, CB_BATT : CB_BATT + 1]

            ones_col = consts.tile([128, 1], f32)
            nc.vector.memset(ones_col, 1.0)

            # ---- preamble: fproj[b, A] per t = (frame @ Wf)[t] ----
            fproj_sb = consts.tile([64, T_LOC, 128], bf16)
            for t in range(T_LOC):
                pf = psmall.tile([64, 128], f32, tag="s", name=f"pf{t}")
                for j in range(NJ):
                    nc.tensor.matmul(
                        pf,
                        lhsT=frameT_sb(j)[:, t * 64 : (t + 1) * 64],
                        rhs=wf_sb(j),
                        start=(j == 0),
                        stop=(j == NJ - 1),
                    )
                nc.scalar.copy(out=fproj_sb[:, t, :], in_=pf)

            # ---- per-timestep body ----
            def body(_iv=None, copies=1):
                per = [({}, {}, {}, []) for _ in range(copies)]

                def load_rT(st, t):
                    rTs, rNs, states, osbs = st
                    rcw = rcws[t]
                    rT = rtp.tile([128, NJ, rcw], fp8, tag="rT", name=f"rT{t}")
                    rTd = regionTc.ap()[:, oT[t] : oT[t] + NJ * rcw].rearrange(
                        "p (j r) -> p j r", j=NJ
                    )
                    nc.sync.dma_start(out=rT, in_=rTd)
                    rTs[t] = rT

                def load_rN(st, t):
                    rTs, rNs, states, osbs = st
                    rN = rnp.tile([128, nchc, 512], bf16, tag="rN", name=f"rN{t}")
                    rNd = regionNc.ap()[t].rearrange("p (c d) -> p c d", c=nchc)
                    # tail chunk is mostly zero-padding rows: ship only the
                    # real ones; zero the rest with DVE memsets (32-wide
                    # pieces for the partition-access rule)
                    ktail = min(128, max(0, rcws[t] - 128 * (nchc - 1)))
                    kload = min(128, -(-ktail // 32) * 32)
                    if kload < 128:
                        for pz in range(kload, 128, 32):
                            nc.vector.memset(rN[pz : pz + 32, nchc - 1, :], 0)
                        nc.sync.dma_start(
                            out=rN[:, : nchc - 1, :], in_=rNd[:, : nchc - 1, :]
                        )
                        if kload > 0:
                            nc.sync.dma_start(
                                out=rN[:kload, nchc - 1, :],
                                in_=rNd[:kload, nchc - 1, :],
                            )
                    else:
                        nc.sync.dma_start(out=rN, in_=rNd)
                    rNs[t] = rN

                def p1(st, t):
                    rTs, rNs, states, osbs = st
                    # phase 1: hidden^T[A, rows] = Wr^T @ region^T (fp8
                    # DoubleRow) + fproj routed via ind2; tanh adds b_att.
                    # jp-outer: one LDWEIGHTS per contraction slice, all
                    # PSUM groups streamed under the same weights.
                    # Matmuls only stream the exact rcw columns; the ghost
                    # zone [rcw, rc) of the last PSUM group is memset so
                    # tanh writes finite junk there (nullified by indB=0).
                    rT = rTs[t]
                    rcw = rcws[t]
                    gs = _groups(rc)
                    mws = [max(0, min(cw, rcw - c0)) for c0, cw in gs]
                    phs = [
                        phh.tile([128, 512], f32, tag="phh", name=f"ph{t}_{g}")
                        for g in range(len(gs))
                    ]
                    for jp in range(2):
                        for g, (c0, cw) in enumerate(gs):
                            if mws[g] == 0:
                                continue
                            nc.tensor.matmul(
                                phs[g][:, : mws[g]],
                                lhsT=w8[:, jp],
                                rhs=rT[:, 2 * jp : 2 * jp + 2, c0 : c0 + mws[g]],
                                start=(jp == 0),
                                stop=False,
                                perf_mode=mybir.MatmulPerfMode.DoubleRow,
                            )
                    for g, (c0, cw) in enumerate(gs):
                        if mws[g] == 0:
                            continue
                        nc.tensor.matmul(
                            phs[g][:, : mws[g]],
                            lhsT=fproj_sb[:, t, :],
                            rhs=ind2_sb[:, t, c0 : c0 + mws[g]],
                            start=False,
                            stop=True,
                        )
                    th = thp.tile([128, rc], bf16, tag="th", name=f"th{t}")
                    for g, (c0, cw) in enumerate(gs):
                        if mws[g] < cw:
                            nc.vector.memset(phs[g][:, mws[g] : cw], 0.0)
                        nc.scalar.activation(
                            out=th[:, c0 : c0 + cw],
                            in_=phs[g][:, :cw],
                            func=AF.Tanh,
                            bias=batt_col,
                        )
                    # att columns + masked-softmax denominator
                    patt = psmall.tile([128, nchc], f32, tag="s", name=f"pa{t}")
                    for c in range(nchc):
                        nc.tensor.matmul(
                            patt[:, c : c + 1],
                            lhsT=th[:, c * 128 : (c + 1) * 128],
                            rhs=wfull_sb,
                            start=True,
                            stop=True,
                        )
                    expf = smallp.tile([128, nchc], f32, tag="expf", name=f"ef{t}")
                    nc.scalar.activation(out=expf, in_=patt, func=AF.Exp)
                    states[t] = expf

                def p2(st, t):
                    rTs, rNs, states, osbs = st
                    # phase 2: out[b, :] = (sum_c diag-expand(e)_c^T @ rN_c)/S
                    expf = states[t]
                    rN = rNs[t]
                    # all nchc diag blocks in one DVE op (broadcast AP)
                    dg = diagp.tile([128, nchc, 64], bf16, tag="dg", name=f"dg{t}")
                    nc.vector.tensor_mul(
                        out=dg,
                        in0=indB_sb[:, t],
                        in1=expf[:, :, None].broadcast_to([128, nchc, 64]),
                    )
                    # denominator: fold dg over chunks (strided DVE
                    # reduce), then one ones-matmul folds partitions
                    srows = smallp.tile([128, 64], f32, tag="sr", name=f"sr{t}")
                    nc.vector.tensor_reduce(
                        out=srows[:, :, None],
                        in_=dg.transpose([0, 2, 1]),
                        axis=mybir.AxisListType.X,
                        op=mybir.AluOpType.add,
                    )
                    po_t = po.tile([64, 512], f32, tag="po", name=f"po{t}")
                    for c in range(nchc):
                        nc.tensor.matmul(
                            po_t,
                            lhsT=dg[:, c, :],
                            rhs=rN[:, c, :],
                            start=(c == 0),
                            stop=(c == nchc - 1),
                        )
                    psS = psmall.tile([64, 1], f32, tag="s", name=f"ps{t}")
                    nc.tensor.matmul(psS, lhsT=srows, rhs=ones_col, start=True, stop=True)
                    rs = smallp.tile([64, 1], f32, tag="rs", name=f"rs{t}")
                    nc.vector.reciprocal(out=rs, in_=psS)
                    osb = outp.tile([64, 512], bf16, tag="osb", name=f"ob{t}")
                    nc.scalar.activation(out=osb, in_=po_t, func=AF.Copy, scale=rs)
                    # store on the ACT HWDGE ring: load FIFO stays clean
                    nc.scalar.dma_start(out=out.ap()[t], in_=osb)

                for u, st in enumerate(per):
                    for t in range(T_LOC):
                        load_rT(st, t)
                    for t in range(T_LOC):
                        load_rN(st, t)
                for st in per:
                    for t in range(T_LOC):
                        p1(st, t)
                    for t in range(T_LOC):
                        p2(st, t)

            if iters == 1:
                body()
            else:
                assert iters % unroll == 0, (iters, unroll)
                with tc.For_i(
                    0,
                    iters // unroll,
                    1,
                    hint_engines=(mybir.EngineType.PE,),
                    staggered_reset=True,
                ) as iv:
                    body(iv, copies=unroll)

    nc.compile()
    return nc


def _get_nc(iters=1, nchc=10, unroll=None, rcws=None):
    if unroll is None:
        unroll = 16 if iters > 1 and iters % 16 == 0 else 1
    key = (iters, nchc, unroll, rcws)
    if key not in _NC_CACHE:
        _NC_CACHE[key] = _build_nc(iters, nchc, unroll, rcws)
    return _NC_CACHE[key]


def _nchc_for(mask):
    keep = ~np.asarray(mask, bool).reshape(T, ROWS)
    counts = keep.sum(axis=1)
    return max(1, int(-(-int(counts.max()) // 128)))


def _rcws_for(mask, nchc=None):
    """Exact per-t-slot row width: max kept rows over cores, ceil-16."""
    if nchc is None:
        nchc = _nchc_for(mask)
    rc = nchc * 128
    keep = ~np.asarray(mask, bool).reshape(N_CORES, T_LOC, ROWS)
    counts = keep.sum(axis=2).max(axis=0)  # [T_LOC]
    return tuple(int(min(rc, -(-int(c) // 16) * 16)) for c in counts)


def _make_in_maps(
    region_feat, frame_feat, mask, W_att, b_att, W_full, nchc=None, rcws=None
):
    bf16 = ml_dtypes.bfloat16
    fp8 = ml_dtypes.float8_e4m3
    mask = np.asarray(mask, bool)
    if nchc is None:
        nchc = _nchc_for(mask)
    if rcws is None:
        rcws = _rcws_for(mask, nchc)
    rc = nchc * 128

    region_f = np.asarray(region_feat, np.float32)        # [T, N, B, D]
    frame_b = np.asarray(frame_feat).astype(bf16)         # [T, B, D]

    w8 = np.ascontiguousarray(
        np.asarray(W_att)[:512]
        .astype(fp8)
        .reshape(4, 128, 128)
        .transpose(1, 0, 2)
        .reshape(128, 512)
    )

    in_maps = []
    for cidx in range(N_CORES):
        sl = slice(cidx * T_LOC, (cidx + 1) * T_LOC)
        regc = np.zeros((T_LOC, rc, D), np.float32)
        ohe = np.zeros((T_LOC, rc, 64), np.float32)
        for tt in range(T_LOC):
            kept = np.flatnonzero(~mask[sl][tt].reshape(ROWS))
            nk = len(kept)
            regc[tt, :nk] = region_f[sl][tt].reshape(ROWS, D)[kept]
            ohe[tt, np.arange(nk), kept % 64] = 1.0
        regT = np.concatenate(
            [
                regc[tt, : rcws[tt]]
                .astype(fp8)
                .reshape(rcws[tt], NJ, 128)
                .transpose(2, 1, 0)
                .reshape(128, NJ * rcws[tt])
                for tt in range(T_LOC)
            ],
            axis=1,
        )
        regT = np.ascontiguousarray(regT)
        regN = np.ascontiguousarray(
            regc.astype(bf16)
            .reshape(T_LOC, nchc, 128, 512)
            .transpose(0, 2, 1, 3)
            .reshape(T_LOC, 128, nchc * 512)
        )
        iB = np.ascontiguousarray(
            ohe.reshape(T_LOC, nchc, 128, 64)
            .transpose(2, 0, 1, 3)
            .reshape(128, T_LOC * nchc * 64)
        ).astype(fp8)
        i2 = np.ascontiguousarray(
            ohe.transpose(2, 0, 1).reshape(64, T_LOC * rc)
        ).astype(fp8)
        frm = frame_b[sl].reshape(T_LOC * B, NJ, 128)     # [tb, j, dd]
        frmT = frm.transpose(2, 1, 0).reshape(128, NJ * 256)
        cb = np.zeros((128, CB_W), bf16)
        cb[:, CB_WF : CB_WF + 512] = (
            np.asarray(W_att)[512:].reshape(4, 128, 128).transpose(1, 0, 2).reshape(128, 512)
        ).astype(bf16)
        cb[:, CB_WFULL] = np.asarray(W_full).astype(bf16)
        cb[:, CB_FRAMET : CB_FRAMET + 1024] = frmT
        cb[:, CB_BATT] = np.asarray(b_att).astype(bf16)
        in_maps.append(
            {
                "regionTc": regT,
                "regionNc": regN,
                "cb16": np.ascontiguousarray(cb),
                "watt8": w8,
                "indB": iB,
                "ind2": i2,
            }
        )
    return in_maps


def kernel(region_feat, frame_feat, mask, W_att, b_att, W_full, b_full=None):
    """Full-input entry point. b_full is accepted but unused: softmax is
    invariant to a constant shift of the logits."""
    from concourse.bass_utils import run_bass_kernel_spmd

    nchc = _nchc_for(mask)
    rcws = _rcws_for(mask, nchc)
    nc = _get_nc(1, nchc, rcws=rcws)
    in_maps = _make_in_maps(
        region_feat, frame_feat, mask, W_att, b_att, W_full, nchc=nchc, rcws=rcws
    )
    res = run_bass_kernel_spmd(nc, in_maps, core_ids=list(range(N_CORES)))
    return np.concatenate(
        [res.results[c]["out"] for c in range(N_CORES)], axis=0
    ).astype(np.float32)
